# revision 7
# baseline (speedup 1.0000x reference)
"""Brute-force KNN density estimator on 8 Trainium2 NeuronCores.

reference math:
    dist[i, j] = ||x_i - x_j||_2 over features [8192, 1024]
    kth[i] = 6th smallest of dist[i, :]  (self-distance included)
    out[i] = 1 / (kth[i] + 1e-8)

Strategy (data-parallel over query rows, 1024 rows per core):
    - TensorE: G = Q @ F^T in bf16 (fp32 PSUM accumulation).
    - Ranking per row i over j only needs T[i,j] = 2*G[i,j] - sq[j]
      (sq[i] is constant per row; sqrt is monotonic) -> one fused
      scalar_tensor_tensor from PSUM, then the DVE `max` (top-8) per
      512-wide tile. Final top-8 over the per-tile candidates gives the
      exact 6th smallest; kth_d2 = sq[i] - T6 recovered with exact fp32
      norms computed on host, so bf16 error only enters via 2*G.
"""

import os

import numpy as np
import ml_dtypes

N = 8192          # points
D = 1024          # feature dim
NCORES = 8
ROWS = N // NCORES   # rows (queries) per core
RT = ROWS // 128     # row tiles per core
CTILE = 512          # matmul moving free dim
CT = N // CTILE      # column tiles
KC = D // 128        # contraction chunks
K_ORD = 5            # 0-based rank -> 6th smallest
EPS = 1e-8

TRACE = bool(int(os.environ.get("KNN_TRACE", "0")))
LAST_EXEC_NS = None


def _build_nc():
    import concourse.mybir as mybir
    from concourse import bacc
    from concourse.tile import TileContext

    dt = mybir.dt
    nc = bacc.Bacc(None, target_bir_lowering=False)

    ft_d = nc.dram_tensor("ft", [CT, KC, 128, CTILE], dt.bfloat16, kind="ExternalInput")
    qt_d = nc.dram_tensor("qt", [KC, 128, ROWS], dt.bfloat16, kind="ExternalInput")
    sq_d = nc.dram_tensor("sq", [128, N], dt.float32, kind="ExternalInput")
    sqi_d = nc.dram_tensor("sqi", [128, RT], dt.float32, kind="ExternalInput")
    out_d = nc.dram_tensor("out", [RT, 128, 1], dt.float32, kind="ExternalOutput")

    with TileContext(nc) as tc:
        with (
            tc.tile_pool(name="persist", bufs=1) as persist,
            tc.tile_pool(name="ftp", bufs=3) as ftp,
            tc.tile_pool(name="tbp", bufs=4) as tbp,
            tc.tile_pool(name="small", bufs=4) as small,
            tc.tile_pool(name="psum", bufs=4, space="PSUM") as psum,
        ):
            qt_s = persist.tile([128, KC, ROWS], dt.bfloat16)
            sq_s = persist.tile([128, N], dt.float32)
            sqi_s = persist.tile([128, RT], dt.float32)
            cand = persist.tile([128, RT * CT * 8], dt.float32)

            for k in range(KC):
                nc.sync.dma_start(qt_s[:, k, :], qt_d[k])
            # chunked so each column-tile's consumer waits on a single DMA sem
            for t in range(CT):
                nc.sync.dma_start(
                    sq_s[:, t * CTILE:(t + 1) * CTILE],
                    sq_d[:, t * CTILE:(t + 1) * CTILE],
                )
            nc.sync.dma_start(sqi_s, sqi_d[:, :])

            for t in range(CT):
                ft_t = ftp.tile([128, KC, CTILE], dt.bfloat16, tag="ft")
                for k in range(KC):
                    nc.sync.dma_start(ft_t[:, k, :], ft_d[t, k])
                for r in range(RT):
                    ps = psum.tile([128, CTILE], dt.float32, tag="ps")
                    for k in range(KC):
                        nc.tensor.matmul(
                            ps,
                            lhsT=qt_s[:, k, r * 128:(r + 1) * 128],
                            rhs=ft_t[:, k, :],
                            start=(k == 0),
                            stop=(k == KC - 1),
                        )
                    # ft is pre-scaled by 2 on host, so ps already holds 2*G
                    tbuf = tbp.tile([128, CTILE], dt.float32, tag="tbuf")
                    nc.vector.tensor_sub(
                        tbuf, ps, sq_s[:, t * CTILE:(t + 1) * CTILE]
                    )
                    nc.vector.max(
                        out=cand[:, (r * CT + t) * 8:(r * CT + t + 1) * 8],
                        in_=tbuf,
                    )

            for r in range(RT):
                top8 = small.tile([128, 8], dt.float32, tag="top8")
                nc.vector.max(out=top8, in_=cand[:, r * CT * 8:(r + 1) * CT * 8])
                kd = small.tile([128, 1], dt.float32, tag="kd")
                nc.vector.tensor_sub(kd, sqi_s[:, r:r + 1], top8[:, K_ORD:K_ORD + 1])
                nc.vector.tensor_scalar_max(kd, kd, 0.0)
                ks = small.tile([128, 1], dt.float32, tag="ks")
                nc.scalar.activation(ks, kd, mybir.ActivationFunctionType.Sqrt)
                nc.vector.tensor_scalar_add(ks, ks, EPS)
                dens = small.tile([128, 1], dt.float32, tag="dens")
                nc.vector.reciprocal(dens, ks)
                nc.sync.dma_start(out_d[r], dens)

    # run Bacc's passes (register allocation, event-semaphore wait splitting)
    # before handing off to the PJRT path, which binds without finalizing
    nc.finalize()
    return nc


def kernel(features):
    global LAST_EXEC_NS
    from concourse.bass_utils import run_bass_kernel_spmd

    f32 = np.ascontiguousarray(np.asarray(features, dtype=np.float32))
    assert f32.shape == (N, D)

    sq = np.einsum("nd,nd->n", f32, f32, dtype=np.float32)   # exact fp32 norms
    ftb = f32.T.astype(ml_dtypes.bfloat16)                    # [D, N]
    # moving operand pre-scaled by 2 (exact in bf16) so PSUM accumulates 2*G
    ft2 = (ftb.astype(np.float32) * 2.0).astype(ml_dtypes.bfloat16)
    # [D, N] -> [CT, KC, 128, CTILE] so each column tile is one contiguous DMA
    ft_tiles = np.ascontiguousarray(
        ft2.reshape(KC, 128, CT, CTILE).transpose(2, 0, 1, 3)
    )
    sq_rep = np.ascontiguousarray(np.broadcast_to(sq, (128, N)))

    in_maps = []
    for c in range(NCORES):
        lo = c * ROWS
        qt = np.ascontiguousarray(ftb[:, lo:lo + ROWS].reshape(KC, 128, ROWS))
        sqi = np.ascontiguousarray(sq[lo:lo + ROWS].reshape(RT, 128).T)
        in_maps.append({"ft": ft_tiles, "qt": qt, "sq": sq_rep, "sqi": sqi})

    nc = _build_nc()
    res = run_bass_kernel_spmd(nc, in_maps, core_ids=list(range(NCORES)), trace=TRACE)
    LAST_EXEC_NS = res.exec_time_ns

    out = np.concatenate([r["out"].reshape(-1) for r in res.results])
    return out.astype(np.float32)[:, None]


# revision 8
# speedup vs baseline: 1.3871x; 1.3871x over previous
"""Brute-force KNN density estimator on 8 Trainium2 NeuronCores.

reference math:
    dist[i, j] = ||x_i - x_j||_2 over features [8192, 1024]
    kth[i] = 6th smallest of dist[i, :]  (self-distance included)
    out[i] = 1 / (kth[i] + 1e-8)

Strategy (data-parallel over query rows, 1024 rows per core):
    - Rank rows of the distance matrix by T[i,j] = 2*G[i,j] - (sq[j] - mean(sq))
      (per-row-constant sq[i] and the monotone sqrt don't change ranking).
    - TensorE: G via fp8 e4m3 DoubleRow matmuls (2x MAC throughput, fp32
      PSUM accumulation); the norm subtraction is folded into the same
      accumulation group as one bf16 matmul with lhsT = -1/128 constant
      and rhs = centered norms replicated across partitions.
    - VectorE: single MAX8 per [128, 512] PSUM tile -> per-tile top-8
      candidates; final MAX8 over candidates gives the exact 6th largest
      T, recovered to a distance with exact fp32 norms on the host side:
      kth_d2 = (sq[i] + mean(sq)) - T6.
"""

import os

import numpy as np
import ml_dtypes

N = 8192          # points
D = 1024          # feature dim
NCORES = 8
ROWS = N // NCORES   # rows (queries) per core
RT = ROWS // 128     # row tiles per core
CTILE = 512          # matmul moving free dim
CT = N // CTILE      # column tiles
KC = D // 128        # 128-row contraction chunks
K_ORD = 5            # 0-based rank -> 6th smallest
EPS = 1e-8
WARMUP_MM = 24       # dummy matmuls to trigger the PE HAM warm clock early

TRACE = bool(int(os.environ.get("KNN_TRACE", "0")))
LAST_EXEC_NS = None


def _build_nc():
    import concourse.mybir as mybir
    from concourse import bacc
    from concourse.tile import TileContext

    dt = mybir.dt
    nc = bacc.Bacc(None, target_bir_lowering=False)

    ft_d = nc.dram_tensor("ft", [CT, KC, 128, CTILE], dt.float8e4, kind="ExternalInput")
    qt_d = nc.dram_tensor("qt", [KC, 128, ROWS], dt.float8e4, kind="ExternalInput")
    sqc_d = nc.dram_tensor("sqc", [128, N], dt.bfloat16, kind="ExternalInput")
    sqi_d = nc.dram_tensor("sqi", [128, RT], dt.float32, kind="ExternalInput")
    out_d = nc.dram_tensor("out", [RT, 128, 1], dt.float32, kind="ExternalOutput")

    DR = mybir.MatmulPerfMode.DoubleRow

    with TileContext(nc) as tc:
        with (
            tc.tile_pool(name="persist", bufs=1) as persist,
            tc.tile_pool(name="ftp", bufs=3) as ftp,
            tc.tile_pool(name="small", bufs=4) as small,
            tc.tile_pool(name="psum", bufs=6, space="PSUM") as psum,
        ):
            qt_s = persist.tile([128, KC, ROWS], dt.float8e4)
            sqc_s = persist.tile([128, N], dt.bfloat16)
            sqi_s = persist.tile([128, RT], dt.float32)
            cand = persist.tile([128, RT * CT * 8], dt.float32)
            neg_s = persist.tile([128, 128], dt.bfloat16)
            warm_s = persist.tile([128, CTILE], dt.bfloat16)

            # PE warm-up: keep the PE busy during the initial DMA window so
            # the HAM clock gate reaches 2.4 GHz before the real matmuls
            nc.vector.memset(neg_s, -1.0 / 128.0)
            nc.vector.memset(warm_s, 0.0)
            wps = psum.tile([128, CTILE], dt.float32, tag="ps")
            for i in range(WARMUP_MM):
                nc.tensor.matmul(wps, lhsT=neg_s, rhs=warm_s,
                                 start=(i == 0), stop=(i == WARMUP_MM - 1))

            for k in range(KC):
                nc.sync.dma_start(qt_s[:, k, :], qt_d[k])
            nc.sync.dma_start(sqi_s, sqi_d[:, :])
            # chunked so each column-tile's consumer waits on one DMA
            for t in range(CT):
                nc.sync.dma_start(
                    sqc_s[:, t * CTILE:(t + 1) * CTILE],
                    sqc_d[:, t * CTILE:(t + 1) * CTILE],
                )

            for t in range(CT):
                ft_t = ftp.tile([128, KC, CTILE], dt.float8e4, tag="ft")
                for k in range(KC):
                    nc.sync.dma_start(ft_t[:, k, :], ft_d[t, k])
                sqc_t = sqc_s[:, t * CTILE:(t + 1) * CTILE]
                for r in range(RT):
                    ps = psum.tile([128, CTILE], dt.float32, tag="ps")
                    for k in range(0, KC, 2):
                        nc.tensor.matmul(
                            ps,
                            lhsT=qt_s[:, k:k + 2, r * 128:(r + 1) * 128],
                            rhs=ft_t[:, k:k + 2, :],
                            start=(k == 0),
                            stop=False,
                            perf_mode=DR,
                        )
                    # T = 2G - sqc: rhs replicated across K partitions,
                    # scaled by lhsT = -1/128
                    nc.tensor.matmul(ps, lhsT=neg_s, rhs=sqc_t,
                                     start=False, stop=True)
                    nc.vector.max(
                        out=cand[:, (r * CT + t) * 8:(r * CT + t + 1) * 8],
                        in_=ps,
                    )

            for r in range(RT):
                top8 = small.tile([128, 8], dt.float32, tag="top8")
                nc.vector.max(out=top8, in_=cand[:, r * CT * 8:(r + 1) * CT * 8])
                kd = small.tile([128, 1], dt.float32, tag="kd")
                nc.vector.tensor_sub(kd, sqi_s[:, r:r + 1], top8[:, K_ORD:K_ORD + 1])
                nc.vector.tensor_scalar_max(kd, kd, 0.0)
                ks = small.tile([128, 1], dt.float32, tag="ks")
                nc.scalar.activation(ks, kd, mybir.ActivationFunctionType.Sqrt)
                nc.vector.tensor_scalar_add(ks, ks, EPS)
                dens = small.tile([128, 1], dt.float32, tag="dens")
                nc.vector.reciprocal(dens, ks)
                nc.sync.dma_start(out_d[r], dens)

    # run Bacc's passes (register allocation, event-semaphore wait splitting)
    # before handing off to the PJRT path, which binds without finalizing
    nc.finalize()
    return nc


def kernel(features):
    global LAST_EXEC_NS
    from concourse.bass_utils import run_bass_kernel_spmd

    f32 = np.ascontiguousarray(np.asarray(features, dtype=np.float32))
    assert f32.shape == (N, D)

    sq = np.einsum("nd,nd->n", f32, f32, dtype=np.float32)   # exact fp32 norms
    sbar = float(sq.mean())
    ftq = f32.T.astype(ml_dtypes.float8_e4m3fn)               # [D, N] fp8
    # moving operand pre-scaled by 2 (exact in fp8) so PSUM accumulates 2*G
    ft2 = (ftq.astype(np.float32) * 2.0).astype(ml_dtypes.float8_e4m3fn)
    # [D, N] -> [CT, KC, 128, CTILE] so each column tile is one contiguous DMA
    ft_tiles = np.ascontiguousarray(
        ft2.reshape(KC, 128, CT, CTILE).transpose(2, 0, 1, 3)
    )
    sqc_rep = np.ascontiguousarray(
        np.broadcast_to((sq - sbar).astype(ml_dtypes.bfloat16), (128, N))
    )

    in_maps = []
    for c in range(NCORES):
        lo = c * ROWS
        qt = np.ascontiguousarray(ftq[:, lo:lo + ROWS].reshape(KC, 128, ROWS))
        sqi = np.ascontiguousarray(
            (sq[lo:lo + ROWS] + sbar).reshape(RT, 128).T.astype(np.float32)
        )
        in_maps.append({"ft": ft_tiles, "qt": qt, "sqc": sqc_rep, "sqi": sqi})

    nc = _build_nc()
    res = run_bass_kernel_spmd(nc, in_maps, core_ids=list(range(NCORES)), trace=TRACE)
    LAST_EXEC_NS = res.exec_time_ns

    out = np.concatenate([r["out"].reshape(-1) for r in res.results])
    return out.astype(np.float32)[:, None]


# revision 11
# speedup vs baseline: 1.6233x; 1.1703x over previous
"""Brute-force KNN density estimator on 8 Trainium2 NeuronCores.

reference math:
    dist[i, j] = ||x_i - x_j||_2 over features [8192, 1024]
    kth[i] = 6th smallest of dist[i, :]  (self-distance included)
    out[i] = 1 / (kth[i] + 1e-8)

Strategy (data-parallel over query rows, 1024 rows per core):
    - Rank rows of the distance matrix by T[i,j] = 2*G[i,j] - (sq[j] - mean(sq))
      (per-row-constant sq[i] and the monotone sqrt don't change ranking).
    - TensorE: G via fp8 e4m3 DoubleRow matmuls (2x MAC throughput, fp32
      PSUM accumulation); the norm subtraction is folded into the same
      accumulation group as one bf16 matmul with lhsT = -1/128 constant
      and rhs = centered norms replicated across partitions.
    - VectorE: single MAX8 per [128, 512] PSUM tile -> per-tile top-8
      candidates; final MAX8 over candidates gives the exact 6th largest
      T, recovered to a distance with exact fp32 norms on the host side:
      kth_d2 = (sq[i] + mean(sq)) - T6.
"""

import os

import numpy as np
import ml_dtypes

N = 8192          # points
D = 1024          # feature dim
NCORES = 8
ROWS = N // NCORES   # rows (queries) per core
RT = ROWS // 128     # row tiles per core
CTILE = 512          # matmul moving free dim
CT = N // CTILE      # column tiles
KC = D // 128        # 128-row contraction chunks
K_ORD = 5            # 0-based rank -> 6th smallest
EPS = 1e-8
WARMUP_MM = 40       # dummy matmuls to trigger the PE HAM warm clock early

TRACE = bool(int(os.environ.get("KNN_TRACE", "0")))
LAST_EXEC_NS = None


def _build_nc():
    import concourse.mybir as mybir
    from concourse import bacc
    from concourse.tile import TileContext

    dt = mybir.dt
    nc = bacc.Bacc(None, target_bir_lowering=False)

    # per-tile layout [CT][128 part][KC*CTILE contiguous] -> one DMA per tile
    ft_d = nc.dram_tensor("ft", [CT, 128, KC * CTILE], dt.float8e4, kind="ExternalInput")
    qt_d = nc.dram_tensor("qt", [128, KC * ROWS], dt.float8e4, kind="ExternalInput")
    sqc_d = nc.dram_tensor("sqc", [128, N], dt.bfloat16, kind="ExternalInput")
    sqi_d = nc.dram_tensor("sqi", [128, RT], dt.float32, kind="ExternalInput")
    out_d = nc.dram_tensor("out", [128, RT], dt.float32, kind="ExternalOutput")

    DR = mybir.MatmulPerfMode.DoubleRow

    with TileContext(nc) as tc:
        with (
            tc.tile_pool(name="persist", bufs=1) as persist,
            tc.tile_pool(name="ftp", bufs=3) as ftp,
            tc.tile_pool(name="small", bufs=2) as small,
            tc.tile_pool(name="psum", bufs=6, space="PSUM") as psum,
        ):
            qt_s = persist.tile([128, KC, ROWS], dt.float8e4)
            sqc_s = persist.tile([128, N], dt.bfloat16)
            sqi_s = persist.tile([128, RT], dt.float32)
            cand = persist.tile([128, RT * CT * 8], dt.float32)
            top8s = persist.tile([128, RT, 8], dt.float32)
            neg_s = persist.tile([128, 128], dt.bfloat16)
            warm_s = persist.tile([128, CTILE], dt.bfloat16)

            # PE warm-up: keep the PE busy during the initial DMA window so
            # the HAM clock gate reaches 2.4 GHz before the real matmuls
            nc.vector.memset(neg_s, -1.0 / 128.0)
            nc.vector.memset(warm_s, 0.0)
            wps = psum.tile([128, CTILE], dt.float32, tag="ps")
            for i in range(WARMUP_MM):
                nc.tensor.matmul(wps, lhsT=neg_s, rhs=warm_s,
                                 start=(i == 0), stop=(i == WARMUP_MM - 1))

            ft_tiles = []
            for t in range(2):  # first two column tiles up front
                ft_t = ftp.tile([128, KC, CTILE], dt.float8e4, tag="ft")
                nc.sync.dma_start(ft_t, ft_d[t].rearrange("p (k j) -> p k j", k=KC))
                ft_tiles.append(ft_t)
            nc.sync.dma_start(qt_s, qt_d[:, :].rearrange("p (k i) -> p k i", k=KC))
            nc.sync.dma_start(sqi_s, sqi_d[:, :])
            for t in range(CT):
                nc.sync.dma_start(
                    sqc_s[:, t * CTILE:(t + 1) * CTILE],
                    sqc_d[:, t * CTILE:(t + 1) * CTILE],
                )

            for t in range(CT):
                if t < 2:
                    ft_t = ft_tiles[t]
                else:
                    ft_t = ftp.tile([128, KC, CTILE], dt.float8e4, tag="ft")
                    nc.sync.dma_start(ft_t, ft_d[t].rearrange("p (k j) -> p k j", k=KC))
                sqc_t = sqc_s[:, t * CTILE:(t + 1) * CTILE]
                for r in range(RT):
                    ps = psum.tile([128, CTILE], dt.float32, tag="ps")
                    for k in range(0, KC, 2):
                        nc.tensor.matmul(
                            ps,
                            lhsT=qt_s[:, k:k + 2, r * 128:(r + 1) * 128],
                            rhs=ft_t[:, k:k + 2, :],
                            start=(k == 0),
                            stop=False,
                            perf_mode=DR,
                        )
                    # T = 2G - sqc: rhs replicated across K partitions,
                    # scaled by lhsT = -1/128
                    nc.tensor.matmul(ps, lhsT=neg_s, rhs=sqc_t,
                                     start=False, stop=True)
                    nc.vector.max(
                        out=cand[:, (r * CT + t) * 8:(r * CT + t + 1) * 8],
                        in_=ps,
                    )

            # batched finals: one [128, RT]-wide chain instead of RT chains
            for r in range(RT):
                nc.vector.max(out=top8s[:, r, :],
                              in_=cand[:, r * CT * 8:(r + 1) * CT * 8])
            kd = small.tile([128, RT], dt.float32, tag="kd")
            # T6 column per row-tile: stride-8 slice of top8s
            nc.vector.tensor_sub(kd, sqi_s, top8s[:, :, K_ORD])
            nc.vector.tensor_scalar_max(kd, kd, 0.0)
            ks = small.tile([128, RT], dt.float32, tag="ks")
            nc.scalar.activation(ks, kd, mybir.ActivationFunctionType.Sqrt)
            nc.vector.tensor_scalar_add(ks, ks, EPS)
            dens = small.tile([128, RT], dt.float32, tag="dens")
            nc.vector.reciprocal(dens, ks)
            nc.sync.dma_start(out_d[:, :], dens)

    # run Bacc's passes (register allocation, event-semaphore wait splitting)
    # before handing off to the PJRT path, which binds without finalizing
    nc.finalize()
    return nc


def kernel(features):
    global LAST_EXEC_NS
    from concourse.bass_utils import run_bass_kernel_spmd

    f32 = np.ascontiguousarray(np.asarray(features, dtype=np.float32))
    assert f32.shape == (N, D)

    sq = np.einsum("nd,nd->n", f32, f32, dtype=np.float32)   # exact fp32 norms
    sbar = float(sq.mean())
    ftq = f32.T.astype(ml_dtypes.float8_e4m3fn)               # [D, N] fp8
    # moving operand pre-scaled by 2 (exact in fp8) so PSUM accumulates 2*G
    ft2 = (ftq.astype(np.float32) * 2.0).astype(ml_dtypes.float8_e4m3fn)
    # [D, N] -> [CT, 128, KC*CTILE]: per column tile, partition p holds all
    # KC chunks contiguously -> a single fully-contiguous DMA per tile
    ft_tiles = np.ascontiguousarray(
        ft2.reshape(KC, 128, CT, CTILE).transpose(2, 1, 0, 3).reshape(CT, 128, KC * CTILE)
    )
    sqc_rep = np.ascontiguousarray(
        np.broadcast_to((sq - sbar).astype(ml_dtypes.bfloat16), (128, N))
    )

    in_maps = []
    for c in range(NCORES):
        lo = c * ROWS
        qt = np.ascontiguousarray(
            ftq[:, lo:lo + ROWS].reshape(KC, 128, ROWS).transpose(1, 0, 2).reshape(128, KC * ROWS)
        )
        sqi = np.ascontiguousarray(
            (sq[lo:lo + ROWS] + sbar).reshape(RT, 128).T.astype(np.float32)
        )
        in_maps.append({"ft": ft_tiles, "qt": qt, "sqc": sqc_rep, "sqi": sqi})

    nc = _build_nc()
    res = run_bass_kernel_spmd(nc, in_maps, core_ids=list(range(NCORES)), trace=TRACE)
    LAST_EXEC_NS = res.exec_time_ns

    # out[p, r] = density of global row  c*1024 + r*128 + p
    out = np.concatenate([r["out"].T.reshape(-1) for r in res.results])
    return out.astype(np.float32)[:, None]


# revision 12
# speedup vs baseline: 1.6355x; 1.0075x over previous
"""Brute-force KNN density estimator on 8 Trainium2 NeuronCores.

reference math:
    dist[i, j] = ||x_i - x_j||_2 over features [8192, 1024]
    kth[i] = 6th smallest of dist[i, :]  (self-distance included)
    out[i] = 1 / (kth[i] + 1e-8)

Strategy (data-parallel over query rows, 1024 rows per core):
    - Rank rows of the distance matrix by T[i,j] = 2*G[i,j] - (sq[j] - mean(sq))
      (per-row-constant sq[i] and the monotone sqrt don't change ranking).
    - TensorE: G via fp8 e4m3 DoubleRow matmuls (2x MAC throughput, fp32
      PSUM accumulation); the norm subtraction is folded into the same
      accumulation group as one bf16 matmul with lhsT = -1/128 constant
      and rhs = centered norms replicated across partitions.
    - VectorE: single MAX8 per [128, 512] PSUM tile -> per-tile top-8
      candidates; final MAX8 over candidates gives the exact 6th largest
      T, recovered to a distance with exact fp32 norms on the host side:
      kth_d2 = (sq[i] + mean(sq)) - T6.
"""

import os

import numpy as np
import ml_dtypes

N = 8192          # points
D = 1024          # feature dim
NCORES = 8
ROWS = N // NCORES   # rows (queries) per core
RT = ROWS // 128     # row tiles per core
CTILE = 512          # matmul moving free dim
CT = N // CTILE      # column tiles
KC = D // 128        # 128-row contraction chunks
K_ORD = 5            # 0-based rank -> 6th smallest
EPS = 1e-8
WARMUP_MM = 28       # dummy matmuls to trigger the PE HAM warm clock early

TRACE = bool(int(os.environ.get("KNN_TRACE", "0")))
LAST_EXEC_NS = None


def _build_nc():
    import concourse.mybir as mybir
    from concourse import bacc
    from concourse.tile import TileContext

    dt = mybir.dt
    nc = bacc.Bacc(None, target_bir_lowering=False, enable_partition_id=False)

    # per-tile layout [CT][128 part][KC*CTILE contiguous] -> one DMA per tile
    ft_d = nc.dram_tensor("ft", [CT, 128, KC * CTILE], dt.float8e4, kind="ExternalInput")
    qt_d = nc.dram_tensor("qt", [128, KC * ROWS], dt.float8e4, kind="ExternalInput")
    sqc_d = nc.dram_tensor("sqc", [128, N], dt.bfloat16, kind="ExternalInput")
    sqi_d = nc.dram_tensor("sqi", [128, RT], dt.float32, kind="ExternalInput")
    out_d = nc.dram_tensor("out", [128, RT], dt.float32, kind="ExternalOutput")

    DR = mybir.MatmulPerfMode.DoubleRow

    with TileContext(nc) as tc:
        with (
            tc.tile_pool(name="persist", bufs=1) as persist,
            tc.tile_pool(name="ftp", bufs=3) as ftp,
            tc.tile_pool(name="small", bufs=2) as small,
            tc.tile_pool(name="psum", bufs=8, space="PSUM") as psum,
        ):
            qt_s = persist.tile([128, KC, ROWS], dt.float8e4)
            sqc_s = persist.tile([128, N], dt.bfloat16)
            sqi_s = persist.tile([128, RT], dt.float32)
            cand = persist.tile([128, RT * CT * 8], dt.float32)
            top8s = persist.tile([128, RT, 8], dt.float32)
            neg_s = persist.tile([128, 128], dt.bfloat16)
            warm_s = persist.tile([128, CTILE], dt.bfloat16)

            # PE warm-up: keep the PE busy during the initial DMA window so
            # the HAM clock gate reaches 2.4 GHz before the real matmuls
            nc.vector.memset(neg_s, -1.0 / 128.0)
            nc.vector.memset(warm_s, 0.0)
            wps = psum.tile([128, CTILE], dt.float32, tag="ps")
            for i in range(WARMUP_MM):
                nc.tensor.matmul(wps, lhsT=neg_s, rhs=warm_s,
                                 start=(i == 0), stop=(i == WARMUP_MM - 1))

            ft_tiles = []
            ft_t0 = ftp.tile([128, KC, CTILE], dt.float8e4, tag="ft")
            nc.sync.dma_start(ft_t0, ft_d[0].rearrange("p (k j) -> p k j", k=KC))
            ft_tiles.append(ft_t0)
            nc.sync.dma_start(qt_s, qt_d[:, :].rearrange("p (k i) -> p k i", k=KC))
            for t in range(1, 3):  # prefetch the next two column tiles
                ft_t = ftp.tile([128, KC, CTILE], dt.float8e4, tag="ft")
                nc.sync.dma_start(ft_t, ft_d[t].rearrange("p (k j) -> p k j", k=KC))
                ft_tiles.append(ft_t)
            nc.sync.dma_start(sqi_s, sqi_d[:, :])
            for t in range(CT):
                nc.sync.dma_start(
                    sqc_s[:, t * CTILE:(t + 1) * CTILE],
                    sqc_d[:, t * CTILE:(t + 1) * CTILE],
                )

            for t in range(CT):
                if t < 3:
                    ft_t = ft_tiles[t]
                else:
                    ft_t = ftp.tile([128, KC, CTILE], dt.float8e4, tag="ft")
                    nc.sync.dma_start(ft_t, ft_d[t].rearrange("p (k j) -> p k j", k=KC))
                sqc_t = sqc_s[:, t * CTILE:(t + 1) * CTILE]
                for r in range(RT):
                    ps = psum.tile([128, CTILE], dt.float32, tag="ps")
                    for k in range(0, KC, 2):
                        nc.tensor.matmul(
                            ps,
                            lhsT=qt_s[:, k:k + 2, r * 128:(r + 1) * 128],
                            rhs=ft_t[:, k:k + 2, :],
                            start=(k == 0),
                            stop=False,
                            perf_mode=DR,
                        )
                    # T = 2G - sqc: rhs replicated across K partitions,
                    # scaled by lhsT = -1/128
                    nc.tensor.matmul(ps, lhsT=neg_s, rhs=sqc_t,
                                     start=False, stop=True)
                    nc.vector.max(
                        out=cand[:, (r * CT + t) * 8:(r * CT + t + 1) * 8],
                        in_=ps,
                    )

            # batched finals: one [128, RT]-wide chain instead of RT chains
            for r in range(RT):
                nc.vector.max(out=top8s[:, r, :],
                              in_=cand[:, r * CT * 8:(r + 1) * CT * 8])
            kd = small.tile([128, RT], dt.float32, tag="kd")
            # T6 column per row-tile: stride-8 slice of top8s
            nc.vector.tensor_sub(kd, sqi_s, top8s[:, :, K_ORD])
            nc.vector.tensor_scalar_max(kd, kd, 0.0)
            ks = small.tile([128, RT], dt.float32, tag="ks")
            nc.scalar.activation(ks, kd, mybir.ActivationFunctionType.Sqrt)
            nc.vector.tensor_scalar_add(ks, ks, EPS)
            dens = small.tile([128, RT], dt.float32, tag="dens")
            nc.vector.reciprocal(dens, ks)
            nc.sync.dma_start(out_d[:, :], dens)

    # run Bacc's passes (register allocation, event-semaphore wait splitting)
    # before handing off to the PJRT path, which binds without finalizing
    nc.finalize()
    return nc


def kernel(features):
    global LAST_EXEC_NS
    from concourse.bass_utils import run_bass_kernel_spmd

    f32 = np.ascontiguousarray(np.asarray(features, dtype=np.float32))
    assert f32.shape == (N, D)

    sq = np.einsum("nd,nd->n", f32, f32, dtype=np.float32)   # exact fp32 norms
    sbar = float(sq.mean())
    ftq = f32.T.astype(ml_dtypes.float8_e4m3fn)               # [D, N] fp8
    # moving operand pre-scaled by 2 (exact in fp8) so PSUM accumulates 2*G
    ft2 = (ftq.astype(np.float32) * 2.0).astype(ml_dtypes.float8_e4m3fn)
    # [D, N] -> [CT, 128, KC*CTILE]: per column tile, partition p holds all
    # KC chunks contiguously -> a single fully-contiguous DMA per tile
    ft_tiles = np.ascontiguousarray(
        ft2.reshape(KC, 128, CT, CTILE).transpose(2, 1, 0, 3).reshape(CT, 128, KC * CTILE)
    )
    sqc_rep = np.ascontiguousarray(
        np.broadcast_to((sq - sbar).astype(ml_dtypes.bfloat16), (128, N))
    )

    in_maps = []
    for c in range(NCORES):
        lo = c * ROWS
        qt = np.ascontiguousarray(
            ftq[:, lo:lo + ROWS].reshape(KC, 128, ROWS).transpose(1, 0, 2).reshape(128, KC * ROWS)
        )
        sqi = np.ascontiguousarray(
            (sq[lo:lo + ROWS] + sbar).reshape(RT, 128).T.astype(np.float32)
        )
        in_maps.append({"ft": ft_tiles, "qt": qt, "sqc": sqc_rep, "sqi": sqi})

    nc = _build_nc()
    res = run_bass_kernel_spmd(nc, in_maps, core_ids=list(range(NCORES)), trace=TRACE)
    LAST_EXEC_NS = res.exec_time_ns

    # out[p, r] = density of global row  c*1024 + r*128 + p
    out = np.concatenate([r["out"].T.reshape(-1) for r in res.results])
    return out.astype(np.float32)[:, None]


# revision 13
# speedup vs baseline: 1.6477x; 1.0075x over previous
"""Brute-force KNN density estimator on 8 Trainium2 NeuronCores.

reference math:
    dist[i, j] = ||x_i - x_j||_2 over features [8192, 1024]
    kth[i] = 6th smallest of dist[i, :]  (self-distance included)
    out[i] = 1 / (kth[i] + 1e-8)

Strategy (data-parallel over query rows, 1024 rows per core):
    - Rank rows of the distance matrix by T[i,j] = 2*G[i,j] - (sq[j] - mean(sq))
      (per-row-constant sq[i] and the monotone sqrt don't change ranking).
    - TensorE: G via fp8 e4m3 DoubleRow matmuls (2x MAC throughput, fp32
      PSUM accumulation); the norm subtraction is folded into the same
      accumulation group as one bf16 matmul with lhsT = -1/128 constant
      and rhs = centered norms replicated across partitions.
    - VectorE: single MAX8 per [128, 512] PSUM tile -> per-tile top-8
      candidates; final MAX8 over candidates gives the exact 6th largest
      T, recovered to a distance with exact fp32 norms on the host side:
      kth_d2 = (sq[i] + mean(sq)) - T6.
"""

import os

import numpy as np
import ml_dtypes

N = 8192          # points
D = 1024          # feature dim
NCORES = 8
ROWS = N // NCORES   # rows (queries) per core
RT = ROWS // 128     # row tiles per core
CTILE = 512          # matmul moving free dim
CT = N // CTILE      # column tiles
KC = D // 128        # 128-row contraction chunks
K_ORD = 5            # 0-based rank -> 6th smallest
EPS = 1e-8
WARMUP_MM = 22       # dummy matmuls to trigger the PE HAM warm clock early

TRACE = bool(int(os.environ.get("KNN_TRACE", "0")))
LAST_EXEC_NS = None


def _build_nc():
    import concourse.mybir as mybir
    from concourse import bacc
    from concourse.tile import TileContext

    dt = mybir.dt
    nc = bacc.Bacc(None, target_bir_lowering=False, enable_partition_id=False)

    # per-tile layout [CT][128 part][KC*CTILE contiguous] -> one DMA per tile
    ft_d = nc.dram_tensor("ft", [CT, 128, KC * CTILE], dt.float8e4, kind="ExternalInput")
    qt_d = nc.dram_tensor("qt", [128, KC * ROWS], dt.float8e4, kind="ExternalInput")
    sqc_d = nc.dram_tensor("sqc", [128, N], dt.bfloat16, kind="ExternalInput")
    sqi_d = nc.dram_tensor("sqi", [128, RT], dt.float32, kind="ExternalInput")
    out_d = nc.dram_tensor("out", [128, RT], dt.float32, kind="ExternalOutput")

    DR = mybir.MatmulPerfMode.DoubleRow

    with TileContext(nc) as tc:
        with (
            tc.tile_pool(name="persist", bufs=1) as persist,
            tc.tile_pool(name="ftp", bufs=3) as ftp,
            tc.tile_pool(name="small", bufs=2) as small,
            tc.tile_pool(name="psum", bufs=8, space="PSUM") as psum,
        ):
            qt_s = persist.tile([128, KC, ROWS], dt.float8e4)
            sqc_s = persist.tile([128, N], dt.bfloat16)
            sqi_s = persist.tile([128, RT], dt.float32)
            cand = persist.tile([128, RT * CT * 8], dt.float32)
            top8s = persist.tile([128, RT, 8], dt.float32)
            neg_s = persist.tile([128, 128], dt.bfloat16)
            warm_s = persist.tile([128, CTILE], dt.bfloat16)

            # PE warm-up: keep the PE busy during the initial DMA window so
            # the HAM clock gate reaches 2.4 GHz before the real matmuls
            nc.vector.memset(neg_s, -1.0 / 128.0)
            nc.vector.memset(warm_s, 0.0)
            wps = psum.tile([128, CTILE], dt.float32, tag="ps")
            for i in range(WARMUP_MM):
                nc.tensor.matmul(wps, lhsT=neg_s, rhs=warm_s,
                                 start=(i == 0), stop=(i == WARMUP_MM - 1))

            ft_tiles = []
            ft_t0 = ftp.tile([128, KC, CTILE], dt.float8e4, tag="ft")
            nc.sync.dma_start(ft_t0, ft_d[0].rearrange("p (k j) -> p k j", k=KC))
            ft_tiles.append(ft_t0)
            nc.sync.dma_start(qt_s, qt_d[:, :].rearrange("p (k i) -> p k i", k=KC))
            for t in range(1, 3):  # prefetch the next two column tiles
                ft_t = ftp.tile([128, KC, CTILE], dt.float8e4, tag="ft")
                nc.sync.dma_start(ft_t, ft_d[t].rearrange("p (k j) -> p k j", k=KC))
                ft_tiles.append(ft_t)
            nc.sync.dma_start(sqi_s, sqi_d[:, :])
            for t in range(CT):
                nc.sync.dma_start(
                    sqc_s[:, t * CTILE:(t + 1) * CTILE],
                    sqc_d[:, t * CTILE:(t + 1) * CTILE],
                )

            for t in range(CT):
                if t < 3:
                    ft_t = ft_tiles[t]
                else:
                    ft_t = ftp.tile([128, KC, CTILE], dt.float8e4, tag="ft")
                    nc.sync.dma_start(ft_t, ft_d[t].rearrange("p (k j) -> p k j", k=KC))
                sqc_t = sqc_s[:, t * CTILE:(t + 1) * CTILE]
                for r in range(RT):
                    ps = psum.tile([128, CTILE], dt.float32, tag="ps")
                    for k in range(0, KC, 2):
                        nc.tensor.matmul(
                            ps,
                            lhsT=qt_s[:, k:k + 2, r * 128:(r + 1) * 128],
                            rhs=ft_t[:, k:k + 2, :],
                            start=(k == 0),
                            stop=False,
                            perf_mode=DR,
                        )
                    # T = 2G - sqc: rhs replicated across K partitions,
                    # scaled by lhsT = -1/128
                    nc.tensor.matmul(ps, lhsT=neg_s, rhs=sqc_t,
                                     start=False, stop=True)
                    nc.vector.max(
                        out=cand[:, (r * CT + t) * 8:(r * CT + t + 1) * 8],
                        in_=ps,
                    )

            # batched finals: one [128, RT]-wide chain instead of RT chains
            for r in range(RT):
                nc.vector.max(out=top8s[:, r, :],
                              in_=cand[:, r * CT * 8:(r + 1) * CT * 8])
            kd = small.tile([128, RT], dt.float32, tag="kd")
            # T6 column per row-tile: stride-8 slice of top8s
            nc.vector.tensor_sub(kd, sqi_s, top8s[:, :, K_ORD])
            nc.vector.tensor_scalar_max(kd, kd, 0.0)
            ks = small.tile([128, RT], dt.float32, tag="ks")
            nc.scalar.activation(ks, kd, mybir.ActivationFunctionType.Sqrt)
            nc.vector.tensor_scalar_add(ks, ks, EPS)
            dens = small.tile([128, RT], dt.float32, tag="dens")
            nc.vector.reciprocal(dens, ks)
            nc.sync.dma_start(out_d[:, :], dens)

    # run Bacc's passes (register allocation, event-semaphore wait splitting)
    # before handing off to the PJRT path, which binds without finalizing
    nc.finalize()
    return nc


def kernel(features):
    global LAST_EXEC_NS
    from concourse.bass_utils import run_bass_kernel_spmd

    f32 = np.ascontiguousarray(np.asarray(features, dtype=np.float32))
    assert f32.shape == (N, D)

    sq = np.einsum("nd,nd->n", f32, f32, dtype=np.float32)   # exact fp32 norms
    sbar = float(sq.mean())
    ftq = f32.T.astype(ml_dtypes.float8_e4m3fn)               # [D, N] fp8
    # moving operand pre-scaled by 2 (exact in fp8) so PSUM accumulates 2*G
    ft2 = (ftq.astype(np.float32) * 2.0).astype(ml_dtypes.float8_e4m3fn)
    # [D, N] -> [CT, 128, KC*CTILE]: per column tile, partition p holds all
    # KC chunks contiguously -> a single fully-contiguous DMA per tile
    ft_tiles = np.ascontiguousarray(
        ft2.reshape(KC, 128, CT, CTILE).transpose(2, 1, 0, 3).reshape(CT, 128, KC * CTILE)
    )
    sqc_rep = np.ascontiguousarray(
        np.broadcast_to((sq - sbar).astype(ml_dtypes.bfloat16), (128, N))
    )

    in_maps = []
    for c in range(NCORES):
        lo = c * ROWS
        qt = np.ascontiguousarray(
            ftq[:, lo:lo + ROWS].reshape(KC, 128, ROWS).transpose(1, 0, 2).reshape(128, KC * ROWS)
        )
        sqi = np.ascontiguousarray(
            (sq[lo:lo + ROWS] + sbar).reshape(RT, 128).T.astype(np.float32)
        )
        in_maps.append({"ft": ft_tiles, "qt": qt, "sqc": sqc_rep, "sqi": sqi})

    nc = _build_nc()
    res = run_bass_kernel_spmd(nc, in_maps, core_ids=list(range(NCORES)), trace=TRACE)
    LAST_EXEC_NS = res.exec_time_ns

    # out[p, r] = density of global row  c*1024 + r*128 + p
    out = np.concatenate([r["out"].T.reshape(-1) for r in res.results])
    return out.astype(np.float32)[:, None]


# revision 14
# speedup vs baseline: 1.6673x; 1.0119x over previous
"""Brute-force KNN density estimator on 8 Trainium2 NeuronCores.

reference math:
    dist[i, j] = ||x_i - x_j||_2 over features [8192, 1024]
    kth[i] = 6th smallest of dist[i, :]  (self-distance included)
    out[i] = 1 / (kth[i] + 1e-8)

Strategy (data-parallel over query rows, 1024 rows per core):
    - Rank rows of the distance matrix by T[i,j] = 2*G[i,j] - (sq[j] - mean(sq))
      (per-row-constant sq[i] and the monotone sqrt don't change ranking).
    - TensorE: G via fp8 e4m3 DoubleRow matmuls (2x MAC throughput, fp32
      PSUM accumulation); the norm subtraction is folded into the same
      accumulation group as one bf16 matmul with lhsT = -1/128 constant
      and rhs = centered norms replicated across partitions.
    - VectorE: single MAX8 per [128, 512] PSUM tile -> per-tile top-8
      candidates; final MAX8 over candidates gives the exact 6th largest
      T, recovered to a distance with exact fp32 norms on the host side:
      kth_d2 = (sq[i] + mean(sq)) - T6.
"""

import os

import numpy as np
import ml_dtypes

N = 8192          # points
D = 1024          # feature dim
NCORES = 8
ROWS = N // NCORES   # rows (queries) per core
RT = ROWS // 128     # row tiles per core
CTILE = 512          # matmul moving free dim
CT = N // CTILE      # column tiles
KC = D // 128        # 128-row contraction chunks
K_ORD = 5            # 0-based rank -> 6th smallest
EPS = 1e-8
WARMUP_MM = 18       # dummy matmuls to trigger the PE HAM warm clock early

TRACE = bool(int(os.environ.get("KNN_TRACE", "0")))
LAST_EXEC_NS = None


def _build_nc():
    import concourse.mybir as mybir
    from concourse import bacc
    from concourse.tile import TileContext

    dt = mybir.dt
    nc = bacc.Bacc(None, target_bir_lowering=False, enable_partition_id=False)

    # per-tile layout [CT][128 part][KC*CTILE contiguous] -> one DMA per tile
    ft_d = nc.dram_tensor("ft", [CT, 128, KC * CTILE], dt.float8e4, kind="ExternalInput")
    qt_d = nc.dram_tensor("qt", [128, KC * ROWS], dt.float8e4, kind="ExternalInput")
    sqc_d = nc.dram_tensor("sqc", [128, N], dt.bfloat16, kind="ExternalInput")
    sqi_d = nc.dram_tensor("sqi", [128, RT], dt.float32, kind="ExternalInput")
    out_d = nc.dram_tensor("out", [128, RT], dt.float32, kind="ExternalOutput")

    DR = mybir.MatmulPerfMode.DoubleRow

    with TileContext(nc) as tc:
        with (
            tc.tile_pool(name="persist", bufs=1) as persist,
            tc.tile_pool(name="ftp", bufs=3) as ftp,
            tc.tile_pool(name="small", bufs=2) as small,
            tc.tile_pool(name="psum", bufs=8, space="PSUM") as psum,
        ):
            qt_s = persist.tile([128, KC, ROWS], dt.float8e4)
            sqc_s = persist.tile([128, N], dt.bfloat16)
            sqi_s = persist.tile([128, RT], dt.float32)
            cand = persist.tile([128, RT * CT * 8], dt.float32)
            top8s = persist.tile([128, RT, 8], dt.float32)
            neg_s = persist.tile([128, 128], dt.bfloat16)
            warm_s = persist.tile([128, CTILE], dt.bfloat16)

            # PE warm-up: keep the PE busy during the initial DMA window so
            # the HAM clock gate reaches 2.4 GHz before the real matmuls
            nc.vector.memset(neg_s, -1.0 / 128.0)
            nc.vector.memset(warm_s, 0.0)
            wps = psum.tile([128, CTILE], dt.float32, tag="ps")
            for i in range(WARMUP_MM):
                nc.tensor.matmul(wps, lhsT=neg_s, rhs=warm_s,
                                 start=(i == 0), stop=(i == WARMUP_MM - 1))

            ft_tiles = []
            ft_t0 = ftp.tile([128, KC, CTILE], dt.float8e4, tag="ft")
            nc.sync.dma_start(ft_t0, ft_d[0].rearrange("p (k j) -> p k j", k=KC))
            ft_tiles.append(ft_t0)
            nc.sync.dma_start(qt_s, qt_d[:, :].rearrange("p (k i) -> p k i", k=KC))
            for t in range(1, 3):  # prefetch the next two column tiles
                ft_t = ftp.tile([128, KC, CTILE], dt.float8e4, tag="ft")
                nc.sync.dma_start(ft_t, ft_d[t].rearrange("p (k j) -> p k j", k=KC))
                ft_tiles.append(ft_t)
            nc.sync.dma_start(sqi_s, sqi_d[:, :])
            for t in range(CT):
                nc.sync.dma_start(
                    sqc_s[:, t * CTILE:(t + 1) * CTILE],
                    sqc_d[:, t * CTILE:(t + 1) * CTILE],
                )

            for t in range(CT):
                if t < 3:
                    ft_t = ft_tiles[t]
                else:
                    ft_t = ftp.tile([128, KC, CTILE], dt.float8e4, tag="ft")
                    nc.sync.dma_start(ft_t, ft_d[t].rearrange("p (k j) -> p k j", k=KC))
                sqc_t = sqc_s[:, t * CTILE:(t + 1) * CTILE]
                for r in range(RT):
                    ps = psum.tile([128, CTILE], dt.float32, tag="ps")
                    for k in range(0, KC, 2):
                        nc.tensor.matmul(
                            ps,
                            lhsT=qt_s[:, k:k + 2, r * 128:(r + 1) * 128],
                            rhs=ft_t[:, k:k + 2, :],
                            start=(k == 0),
                            stop=False,
                            perf_mode=DR,
                        )
                    # T = 2G - sqc: rhs replicated across K partitions,
                    # scaled by lhsT = -1/128
                    nc.tensor.matmul(ps, lhsT=neg_s, rhs=sqc_t,
                                     start=False, stop=True)
                    nc.vector.max(
                        out=cand[:, (r * CT + t) * 8:(r * CT + t + 1) * 8],
                        in_=ps,
                    )

            # batched finals: one [128, RT]-wide chain instead of RT chains
            for r in range(RT):
                nc.vector.max(out=top8s[:, r, :],
                              in_=cand[:, r * CT * 8:(r + 1) * CT * 8])
            kd = small.tile([128, RT], dt.float32, tag="kd")
            # T6 column per row-tile: stride-8 slice of top8s
            nc.vector.tensor_sub(kd, sqi_s, top8s[:, :, K_ORD])
            nc.vector.tensor_scalar_max(kd, kd, 0.0)
            ks = small.tile([128, RT], dt.float32, tag="ks")
            nc.scalar.activation(ks, kd, mybir.ActivationFunctionType.Sqrt)
            nc.vector.tensor_scalar_add(ks, ks, EPS)
            dens = small.tile([128, RT], dt.float32, tag="dens")
            nc.vector.reciprocal(dens, ks)
            nc.sync.dma_start(out_d[:, :], dens)

    # run Bacc's passes (register allocation, event-semaphore wait splitting)
    # before handing off to the PJRT path, which binds without finalizing
    nc.finalize()
    return nc


def kernel(features):
    global LAST_EXEC_NS
    from concourse.bass_utils import run_bass_kernel_spmd

    f32 = np.ascontiguousarray(np.asarray(features, dtype=np.float32))
    assert f32.shape == (N, D)

    sq = np.einsum("nd,nd->n", f32, f32, dtype=np.float32)   # exact fp32 norms
    sbar = float(sq.mean())
    ftq = f32.T.astype(ml_dtypes.float8_e4m3fn)               # [D, N] fp8
    # moving operand pre-scaled by 2 (exact in fp8) so PSUM accumulates 2*G
    ft2 = (ftq.astype(np.float32) * 2.0).astype(ml_dtypes.float8_e4m3fn)
    # [D, N] -> [CT, 128, KC*CTILE]: per column tile, partition p holds all
    # KC chunks contiguously -> a single fully-contiguous DMA per tile
    ft_tiles = np.ascontiguousarray(
        ft2.reshape(KC, 128, CT, CTILE).transpose(2, 1, 0, 3).reshape(CT, 128, KC * CTILE)
    )
    sqc_rep = np.ascontiguousarray(
        np.broadcast_to((sq - sbar).astype(ml_dtypes.bfloat16), (128, N))
    )

    in_maps = []
    for c in range(NCORES):
        lo = c * ROWS
        qt = np.ascontiguousarray(
            ftq[:, lo:lo + ROWS].reshape(KC, 128, ROWS).transpose(1, 0, 2).reshape(128, KC * ROWS)
        )
        sqi = np.ascontiguousarray(
            (sq[lo:lo + ROWS] + sbar).reshape(RT, 128).T.astype(np.float32)
        )
        in_maps.append({"ft": ft_tiles, "qt": qt, "sqc": sqc_rep, "sqi": sqi})

    nc = _build_nc()
    res = run_bass_kernel_spmd(nc, in_maps, core_ids=list(range(NCORES)), trace=TRACE)
    LAST_EXEC_NS = res.exec_time_ns

    # out[p, r] = density of global row  c*1024 + r*128 + p
    out = np.concatenate([r["out"].T.reshape(-1) for r in res.results])
    return out.astype(np.float32)[:, None]
